# revision 16
# baseline (speedup 1.0000x reference)
"""Trainium2 Bass kernel for nn_BasicFlow (sparse window attention flow).

Sharding: pure data-parallel over batch B=8 -> one image pair per NeuronCore.
Device computes (per core):
  - 4x conv3x3 (128->128ch, 96x96) as row-strip matmuls over a host-prepadded
    input (bf16, or fp8 DoubleRow with optional error-feedback pass)
  - all 8 shift-variant x 144-window correlation matmuls in bf16; the k-side
    windows are read directly from the conv-output tile via strided APs
    (a 4px wrap halo removes the roll), the q-side is gathered window-major
    (one DVE copy per variant) because the PE stationary operand must be
    contiguous.
Raw correlation volumes go to DRAM; the small softmax/flow/splice/bilinear
tail (~1% of FLOPs) is vectorized numpy on host.
"""

import os

# recover wedged NeuronCores at NRT init (observed transient
# NRT_EXEC_UNIT_UNRECOVERABLE; reset-on-load clears it)
os.environ.setdefault("NEURON_RT_RESET_CORES", "1")

import numpy as np
import ml_dtypes
import bass_rust

import concourse.bass as bass
import concourse.bacc as bacc
import concourse.tile as tile
import concourse.mybir as mybir
from concourse import bass_utils

F32 = mybir.dt.float32
BF16 = mybir.dt.bfloat16
F8 = mybir.dt.float8e4
BFNP = ml_dtypes.bfloat16
F8NP = ml_dtypes.float8_e4m3fn

B = 8
DIM = 128
H = W = 96
P = 8
UP = 4
SCALE = DIM ** -0.5
S1 = S2 = H // P          # 12 windows per axis
NW = S1 * S2              # 144 windows
NV = 8                    # 4 shift variants x 2 directions
SHIFTS = ((0, 0), (0, 4), (4, 0), (4, 4))

XW = 98                   # padded input width (1px conv border)
XN = XW * XW + 4          # flat padded image + 4 elem slack for strip reads
QW = 100                  # conv output tile width (4px wrap halo)
RT = 4                    # conv output rows per psum tile
NRT = H // RT
NSTRIP = RT * XW          # 392: psum strip length per conv tile
DELTA = [dy * XW + dx for dy in range(3) for dx in range(3)]

# conv numerics: 'bf16' | 'fp8' (raw DoubleRow) | 'fp8ef' (input error feedback)
MODE = os.environ.get("BASSFLOW_MODE", "fp8ef")

_COMPILED = {}


def _strip_ap(xin, img, slot, nslots, y0, delta, pair_stride):
    """Moving AP for one conv row-strip: [128, (2,) NSTRIP] fp8/bf16 elements
    starting at flat offset 98*y0+delta of image `img`, slot `slot` (ef).
    pair_stride: None -> plain [128, N]; else DoubleRow [128, 2, N]."""
    base = xin[:]
    part = list(base.ap)[0]
    off = base.offset + (img * nslots + slot) * XN + XW * y0 + delta
    if pair_stride is None:
        dims = [list(part), [1, NSTRIP]]
    else:
        dims = [list(part), [pair_stride, 2], [1, NSTRIP]]
    ap = base.copy()
    ap.ap = bass_rust.VecI64Pair(dims)
    ap.offset = off
    return ap


# --------------------------------------------------------------------------
# Device kernel
# --------------------------------------------------------------------------

def _build_device(mode):
    nc = bacc.Bacc("TRN2", target_bir_lowering=False, debug=False, num_devices=8)

    nslots = 2 if mode == "fp8ef" else 1
    xdt = BF16 if mode == "bf16" else F8
    if mode == "bf16":
        wshape = [DIM, 9, DIM]
    elif mode == "fp8":
        wshape = [DIM, 5, 2, DIM]
    else:
        wshape = [DIM, 9, 2, DIM]

    xin_d = nc.dram_tensor("xin", [DIM, 2 * nslots * XN], xdt,
                           kind="ExternalInput")
    wq_d = nc.dram_tensor("wq", wshape, xdt, kind="ExternalInput")
    wk_d = nc.dram_tensor("wk", wshape, xdt, kind="ExternalInput")
    bq_d = nc.dram_tensor("bq", [DIM, 1], F32, kind="ExternalInput")
    bk_d = nc.dram_tensor("bk", [DIM, 1], F32, kind="ExternalInput")
    # raw correlation volumes, window pairs packed across 128 partitions:
    # [variant*2+dir, par*64+q_pixel, window_pair, k_pixel], window = 2*pair+par
    corr_d = nc.dram_tensor("corr", [NV, 2 * P * P, NW // 2, P * P], BF16,
                            kind="ExternalOutput")

    with tile.TileContext(nc) as tc:
        with (
            tc.tile_pool(name="const", bufs=1) as constp,
            tc.tile_pool(name="xin", bufs=1) as xinp,
            tc.tile_pool(name="qk", bufs=1) as qkp,
            tc.tile_pool(name="qwm", bufs=4) as qwmp,
            tc.tile_pool(name="stage", bufs=5) as stagep,
            tc.tile_pool(name="psum", bufs=8, space="PSUM") as psump,
        ):
            wq_sb = constp.tile(wshape, xdt, tag="wq")
            wk_sb = constp.tile(wshape, xdt, tag="wk")
            bq_sb = constp.tile([DIM, 1], F32, tag="bq")
            bk_sb = constp.tile([DIM, 1], F32, tag="bk")
            nc.sync.dma_start(wq_sb[:], wq_d[:])

            xin = xinp.tile([DIM, 2 * nslots * XN], xdt, tag="xin")
            # 8 chunks per (img, slot), chunk-major so conv q0 starts after
            # the first pair of eighth-chunks lands
            xv = xin[:].rearrange("p (s n) -> p s n", s=2 * nslots)
            xd = xin_d[:].rearrange("p (s n) -> p s n", s=2 * nslots)
            NCH = 8
            QC = XN // NCH
            for c in range(NCH):
                for s in range(2 * nslots):
                    hi = (c + 1) * QC if c < NCH - 1 else XN
                    nc.sync.dma_start(xv[:, s, c * QC:hi],
                                      xd[:, s, c * QC:hi])
                if c == 0:
                    nc.sync.dma_start(bq_sb[:], bq_d[:])
                elif c == 1:
                    nc.sync.dma_start(wk_sb[:], wk_d[:])
                    nc.sync.dma_start(bk_sb[:], bk_d[:])

            # conv output tiles with 4px wrap halo (rows/cols 96:100 = 0:4)
            qe0 = qkp.tile([DIM, QW, QW], BF16, tag="qe0")
            ke2 = qkp.tile([DIM, QW, QW], BF16, tag="ke2")
            qe2 = qkp.tile([DIM, QW, QW], BF16, tag="qe2")
            ke0 = qkp.tile([DIM, QW, QW], BF16, tag="ke0")

            def conv(dst, img, w_sb, b_sb):
                for rt in range(NRT):
                    y0 = rt * RT
                    ps = psump.tile([DIM, NSTRIP], F32, tag="ps")
                    if mode == "bf16":
                        for t in range(9):
                            mov = _strip_ap(xin, img, 0, nslots, y0,
                                            DELTA[t], None)
                            nc.tensor.matmul(ps[:], w_sb[:, t, :], mov,
                                             start=(t == 0), stop=(t == 8))
                    elif mode == "fp8":
                        for p5 in range(5):
                            if p5 < 4:
                                a, b = 2 * p5, 2 * p5 + 1
                                stride = DELTA[b] - DELTA[a]
                            else:
                                a, stride = 8, 1  # partner is zero weights
                            mov = _strip_ap(xin, img, 0, nslots, y0,
                                            DELTA[a], stride)
                            nc.tensor.matmul(
                                ps[:], w_sb[:, p5, :, :], mov,
                                start=(p5 == 0), stop=(p5 == 4),
                                perf_mode=mybir.MatmulPerfMode.DoubleRow)
                    else:  # fp8ef: pair dim selects (X8, E8) image slot
                        for t in range(9):
                            mov = _strip_ap(xin, img, 0, nslots, y0,
                                            DELTA[t], XN)
                            nc.tensor.matmul(
                                ps[:], w_sb[:, t, :, :], mov,
                                start=(t == 0), stop=(t == 8),
                                perf_mode=mybir.MatmulPerfMode.DoubleRow)
                    psv = ps[:].rearrange("p (r c) -> p r c", r=RT)[:, :, 0:W]
                    nc.scalar.activation(
                        dst[:, y0:y0 + RT, 0:W], psv,
                        mybir.ActivationFunctionType.Identity, bias=b_sb[:])

            def halo(t):
                # on gpsimd: keeps the DVE queue free for the qwm gathers
                nc.gpsimd.tensor_copy(t[:, W:QW, 0:W], t[:, 0:4, 0:W])
                nc.gpsimd.tensor_copy(t[:, :, W:QW], t[:, :, 0:4])

            def gather(qe, ry, rx):
                qwm = qwmp.tile([DIM, NW, P * P], BF16, tag="qwm")
                src = qe[:, ry:ry + H, rx:rx + W].rearrange(
                    "p (wy ly) (wx lx) -> p wy wx ly lx", ly=P, lx=P)
                dst = qwm[:].rearrange(
                    "p (wy wx) (ly lx) -> p wy wx ly lx", wx=S2, lx=P)
                nc.vector.tensor_copy(dst, src)
                return qwm

            WG = 16                     # windows per psum bank (fills 2KB)
            NG = NW // WG               # 9 psum groups per variant
            copy_engines = (nc.vector, nc.scalar)
            ci = 0

            def corr_variant(d, v, qwm, ke, last=False):
                nonlocal ci
                ry, rx = SHIFTS[v]
                vd = v * 2 + d
                DB = 3                  # psum groups per output DMA
                for wg in range(NG):
                    ps = psump.tile([2 * P * P, WG // 2, P * P], F32,
                                    tag="ps")
                    if wg % DB == 0:
                        sb = stagep.tile(
                            [2 * P * P, DB * WG // 2, P * P], BF16,
                            tag="corrsb")
                    for wi in range(WG // 2):
                        for par in range(2):
                            w = wg * WG + 2 * wi + par
                            wy, wx = divmod(w, S2)
                            mov = ke[:, ry + wy * P: ry + wy * P + P,
                                     rx + wx * P: rx + wx * P + P]
                            nc.tensor.matmul(
                                ps[64 * par:64 * par + 64, wi, :],
                                qwm[:, w, :], mov, start=True,
                                stop=True, tile_position=(0, 64 * par))
                    # during the d1-gather window the DVE is busy with
                    # gathers; route those copies to the Activation engine
                    if 24 <= ci < 54:
                        eng = nc.scalar
                    else:
                        eng = copy_engines[ci % 2]
                    ci += 1
                    g = wg % DB
                    dst = sb[:, g * (WG // 2):(g + 1) * (WG // 2), :]
                    if eng is nc.scalar:
                        eng.copy(dst, ps[:])
                    else:
                        eng.tensor_copy(dst, ps[:])
                    if last and wg >= NG - DB:
                        # final variant: per-group DMAs drain the tail faster
                        w0 = wg * (WG // 2)
                        nc.sync.dma_start(
                            corr_d[vd, :, w0:w0 + WG // 2, :], dst)
                    elif g == DB - 1:
                        w0 = (wg - DB + 1) * (WG // 2)
                        nc.sync.dma_start(
                            corr_d[vd, :, w0:w0 + DB * (WG // 2), :],
                            sb[:])

            conv(qe0, 0, wq_sb, bq_sb)
            halo(qe0)
            conv(ke2, 1, wk_sb, bk_sb)
            halo(ke2)
            qwm0 = [gather(qe0, ry, rx) for ry, rx in SHIFTS]
            conv(qe2, 1, wq_sb, bq_sb)
            halo(qe2)
            conv(ke0, 0, wk_sb, bk_sb)
            halo(ke0)

            # interleave the direction-1 gathers after each direction-0
            # variant: gather d1-v reuses d0-v's qwm slot (bufs=4), so it can
            # start right as corr d0-v's matmuls retire
            qwm2 = []
            for v in range(4):
                corr_variant(0, v, qwm0[v], ke2)
                qwm2.append(gather(qe2, *SHIFTS[v]))
            for v in range(4):
                corr_variant(1, v, qwm2[v], ke0, last=(v == 3))

    nc.compile()
    return nc


# --------------------------------------------------------------------------
# Host-side input prep + device run
# --------------------------------------------------------------------------

def _pad_flat(img):
    """img [DIM, H, W] f32 -> flat padded [DIM, XN] f32 (zero border+slack)."""
    xp = np.zeros((DIM, XN), np.float32)
    v = xp[:, :XW * XW].reshape(DIM, XW, XW)
    v[:, 1:H + 1, 1:W + 1] = img
    return xp


def _prep_weights(w, mode):
    """w (O,I,3,3) f32 -> device layout per mode."""
    wT = np.ascontiguousarray(
        w.astype(np.float32).transpose(1, 2, 3, 0).reshape(DIM, 9, DIM))
    if mode == "bf16":
        return wT.astype(BFNP)
    w8 = wT.astype(F8NP)
    if mode == "fp8ef":
        out = np.zeros((DIM, 9, 2, DIM), F8NP)
        out[:, :, 0, :] = w8
        out[:, :, 1, :] = w8
        return np.ascontiguousarray(out)
    out = np.zeros((DIM, 5, 2, DIM), F8NP)
    for p5 in range(4):
        out[:, p5, 0, :] = w8[:, 2 * p5, :]
        out[:, p5, 1, :] = w8[:, 2 * p5 + 1, :]
    out[:, 4, 0, :] = w8[:, 8, :]
    return np.ascontiguousarray(out)


def _prep_xin(f0, f2, mode):
    flats = [_pad_flat(f0), _pad_flat(f2)]
    if mode == "bf16":
        return np.ascontiguousarray(
            np.stack(flats, axis=1).reshape(DIM, 2 * XN)).astype(BFNP)
    if mode == "fp8":
        return np.ascontiguousarray(
            np.stack(flats, axis=1).reshape(DIM, 2 * XN)).astype(F8NP)
    slots = []
    for fl in flats:
        x8 = fl.astype(F8NP)
        e8 = (fl - x8.astype(np.float32)).astype(F8NP)
        slots += [x8, e8]
    return np.ascontiguousarray(
        np.stack(slots, axis=1).reshape(DIM, 4 * XN))


def _run_device(feat0, feat2, wq, bq, wk, bk):
    mode = MODE
    if mode not in _COMPILED:
        _COMPILED[mode] = _build_device(mode)
    nc = _COMPILED[mode]

    wqT = _prep_weights(wq, mode)
    wkT = _prep_weights(wk, mode)
    bqc = np.ascontiguousarray(bq.astype(np.float32).reshape(DIM, 1))
    bkc = np.ascontiguousarray(bk.astype(np.float32).reshape(DIM, 1))

    in_maps = []
    for b in range(B):
        in_maps.append({
            "xin": _prep_xin(np.asarray(feat0[b], np.float32),
                             np.asarray(feat2[b], np.float32), mode),
            "wq": wqT, "wk": wkT, "bq": bqc, "bk": bkc,
        })
    trace = bool(int(os.environ.get("BASSFLOW_TRACE", "0")))
    res = bass_utils.run_bass_kernel_spmd(nc, in_maps, core_ids=list(range(B)),
                                          trace=trace)
    if trace:
        print(f"HW exec time: {res.exec_time_ns} ns "
              f"(mean {res.mean_exec_time_ns})")
        if res.instructions_and_trace:
            print("trace path:", res.instructions_and_trace[1])
    corr = np.stack([res.results[b]["corr"] for b in range(B)])
    # [B, NV, par*64+q, pair, k] -> [B, NV, win=2*pair+par, q, k]
    corr = corr.reshape(B, NV, 2, P * P, NW // 2, P * P)
    corr = corr.transpose(0, 1, 4, 2, 3, 5).reshape(B, NV, NW, P * P, P * P)
    return corr.astype(np.float32)


# --------------------------------------------------------------------------
# Host tail: bias/mask + softmax flow pipeline + splice + bilinear upsample
# (numpy port of the reference; ~1% of total FLOPs)
# --------------------------------------------------------------------------

def _bias_index():
    coords = np.stack(np.meshgrid(np.arange(P), np.arange(P),
                                  indexing='ij')).reshape(2, -1)
    rel = (coords[:, :, None] - coords[:, None, :]).transpose(1, 2, 0).copy()
    rel[..., 0] += P - 1
    rel[..., 1] += P - 1
    rel[..., 0] *= 2 * P - 1
    return rel.sum(-1).reshape(-1)


def _pos():
    r = np.arange(P, dtype=np.float32)
    yy, xx = np.meshgrid(r, r, indexing='ij')
    return np.stack([xx, yy])[None].reshape(1, 2, P * P)


def _make_mask(Hp, Wp, sh, sw):
    m = np.zeros((Hp, Wp))
    hs = ((slice(0, -sh * 2), slice(-sh * 2, -sh), slice(-sh, None))
          if sh else (slice(None),))
    ws = ((slice(0, -sw * 2), slice(-sw * 2, -sw), slice(-sw, None))
          if sw else (slice(None),))
    cnt = 0
    for a in hs:
        for b in ws:
            m[a, b] = cnt
            cnt += 1
    win = m.reshape(Hp // P, P, Wp // P, P).transpose(0, 2, 1, 3).reshape(-1, P * P)
    d = win[:, None, :] - win[:, :, None]
    return np.where(d != 0, -10000.0, 0.0).astype(np.float32)


def _softmax(x, axis):
    m = np.max(x, axis=axis, keepdims=True)
    e = np.exp(x - m)
    return e / np.sum(e, axis=axis, keepdims=True)


_MID_IDX = None


def _mid_gather():
    """c[b, (j,k), (h2,w2)] = corr[b, (j+3-h2, k+3-w2), (h2,w2)] (0 if invalid)."""
    global _MID_IDX
    if _MID_IDX is None:
        j, k, h2, w2 = np.meshgrid(np.arange(9), np.arange(9), np.arange(P),
                                   np.arange(P), indexing='ij')
        qy = j + 3 - h2
        qx = k + 3 - w2
        valid = (qy >= 0) & (qy < P) & (qx >= 0) & (qx < P)
        qidx = np.clip(qy, 0, P - 1) * P + np.clip(qx, 0, P - 1)
        kidx = h2 * P + w2
        _MID_IDX = (qidx.reshape(81, 64), kidx.reshape(81, 64),
                    valid.reshape(81, 64))
    return _MID_IDX


def _flow_mid(corr, pos):
    bw = corr.shape[0]
    qidx, kidx, valid = _mid_gather()
    c = corr[:, qidx, kidx] * valid[None]          # (bw, 81, 64)
    n = P + 1
    r = np.arange(0.0, P - 0.5, 0.5)
    yy, xx = np.meshgrid(r, r, indexing='ij')
    CH = P // 2 - 1
    base = np.stack([xx, yy])[None][:, :, CH:2 * P - 1 - CH, CH:2 * P - 1 - CH]
    base = base.reshape(1, 2, n * n).astype(np.float32)
    flow = pos[:, :, None, :] - base[:, :, :, None]          # (1,2,81,64)
    smax = _softmax(c, axis=2)
    fl = np.einsum('bmk,cmk->bcm', smax, flow[0]).reshape(bw, 2, n, n)
    cr = np.sum(c * smax, axis=2).reshape(bw, 1, n, n)
    corr4 = np.concatenate([cr[:, :, :-1, :-1], cr[:, :, :-1, 1:],
                            cr[:, :, 1:, :-1], cr[:, :, 1:, 1:]], axis=1)
    flow4 = np.concatenate([fl[:, :, :-1, :-1], fl[:, :, :-1, 1:],
                            fl[:, :, 1:, :-1], fl[:, :, 1:, 1:]], axis=1)
    corr4 = corr4.transpose(0, 2, 3, 1).reshape(bw, P * P, 4)
    flow4 = flow4.reshape(bw, 4, 2, P, P).transpose(0, 2, 3, 4, 1)
    flow4 = flow4.reshape(bw, 2, P * P, 4) * 2
    smax2 = _softmax(corr4, axis=2)
    out = np.sum(flow4 * smax2[:, None], axis=3)
    return out.reshape(bw, 2, P, P).astype(np.float32)


def _flow_bsd(corr, pos):
    cut = P // 4
    bw = corr.shape[0]
    c = corr.reshape(bw, P, P, P * P)[:, cut:P - cut, cut:P - cut, :]
    L = (P - 2 * cut) ** 2
    c = c.reshape(bw, L, P * P)
    base = _pos().reshape(1, 2, P, P)[:, :, cut:P - cut, cut:P - cut]
    base = base.reshape(1, 2, L)
    flow = pos[:, :, None, :] - base[:, :, :, None]
    smax = _softmax(c, axis=2)
    out = np.einsum('blk,clk->bcl', smax, flow[0])
    return out.reshape(bw, 2, P - 2 * cut, P - 2 * cut).astype(np.float32)


def _splice(f00, f01, f10, f11, factor, Ho, Wo):
    f = np.concatenate([np.concatenate([f00, f01], axis=3),
                        np.concatenate([f10, f11], axis=3)], axis=2)
    bs, kk, hh, ww = f.shape
    b = bs // (S1 * S2)
    f = f.reshape(b, S1, S2, kk, hh, ww).transpose(0, 3, 1, 4, 2, 5)
    f = f.reshape(b, kk, S1 * hh, S2 * ww)
    sft = (P // 4) * factor
    f = np.roll(f, (sft, sft), axis=(2, 3))
    return f[:, :, :Ho * factor, :Wo * factor]


def _resize_mat(in_size, out_size):
    scale = out_size / in_size
    sample = (np.arange(out_size) + 0.5) / scale - 0.5
    x = np.abs(sample[None, :] - np.arange(in_size)[:, None])
    w = np.maximum(0.0, 1.0 - x)
    tot = w.sum(0, keepdims=True)
    return (w / np.where(tot == 0, 1.0, tot)).astype(np.float32)


def _up(x, f):
    b, c, h, w = x.shape
    My = _resize_mat(h, h * f)
    Mx = _resize_mat(w, w * f)
    y = np.einsum('bchw,hH->bcHw', x, My)
    y = np.einsum('bcHw,wW->bcHW', y, Mx)
    return (y * f).astype(np.float32)


def _host_flow(corr_raw, bias_table):
    """corr_raw: (B, NV, NW, 64, 64) raw q.k^T dot products."""
    bias = bias_table.astype(np.float32)[_bias_index()].reshape(
        P * P, P * P, 1).transpose(2, 0, 1)          # (1,64,64)
    pos = _pos()
    masks = {}
    for v, (sh, sw) in enumerate(((0, 0), (0, 4), (4, 0), (4, 4))):
        masks[v] = _make_mask(H, W, sh, sw) if (sh or sw) else None

    f1 = {}
    f0 = {}
    for v in range(4):
        for d in range(2):
            c = corr_raw[:, v * 2 + d].reshape(B * NW, 64, 64) * SCALE + bias
            if masks[v] is not None:
                c = (c.reshape(B, NW, 64, 64) + masks[v][None]).reshape(
                    B * NW, 64, 64)
            f1[(v, d)] = _flow_mid(c, pos)
            f0[(v, d)] = _flow_bsd(c, pos)

    # direction 0: (q0,k2) -> flow12 (mid), flow02 (bsd)
    # direction 1: (q2,k0) -> flow10 (mid), flow20 (bsd)
    flow12 = _splice(f1[(0, 0)], f1[(1, 0)], f1[(2, 0)], f1[(3, 0)], 2, H, W)
    flow02 = _splice(f0[(0, 0)], f0[(1, 0)], f0[(2, 0)], f0[(3, 0)], 1, H, W)
    flow10 = _splice(f1[(0, 1)], f1[(1, 1)], f1[(2, 1)], f1[(3, 1)], 2, H, W)
    flow20 = _splice(f0[(0, 1)], f0[(1, 1)], f0[(2, 1)], f0[(3, 1)], 1, H, W)
    fh, ff = UP // 2, UP
    return (_up(flow10, fh), _up(flow12, fh), _up(flow02, ff), _up(flow20, ff))


def kernel(feat0, feat2, wq, bq, wk, bk, bias_table):
    corr_raw = _run_device(np.asarray(feat0), np.asarray(feat2),
                           np.asarray(wq), np.asarray(bq),
                           np.asarray(wk), np.asarray(bk))
    return _host_flow(corr_raw, np.asarray(bias_table))
